# revision 37
# baseline (speedup 1.0000x reference)
"""Causal self-attention (b=2, t=2048, d=1024, h=16) on 8 trn2 NeuronCores.

Sharding: core c handles batch c//4 and the 4 heads 4*(c%4)..4*(c%4)+3
(data parallel over batch x tensor parallel over heads). Each core
computes x @ w_qkv for its head-slice, causal attention for its heads,
and a partial out-projection y_heads @ w_out[head_rows]; the host sums
the 4 partial bf16 outputs per batch (the tensor-parallel all-reduce).

Design (HW ~184-187us vs ~395us f32r baseline; rel err 5.6e-3, gate 2e-2):
  - bf16 end-to-end; x is pre-transposed on the host so every input
    load is a plain contiguous DMA, spread over the sync/scalar HWDGE
    queues plus the gpsimd SWDGE queue (transpose-mode DMAs on two
    queues raced and corrupted SBUF; all-normal-mode is safe).
  - dummy maskt@I matmuls bridge the initial DMA wait so the PE HAM
    clock-gate is warm (2.4 GHz) when real work arrives.
  - causal mask folded into the S PSUM accumulation as a matmul with a
    constant -BIG upper-triangle (maskT @ I), so exp(scale*(S+mask))=0
    above the diagonal -- no vector-engine masking pass.
  - one fused exp per j-chunk over both heads ([128, 2, 512] PSUM tile);
    the PE emission is software-pipelined (S of chunk n+1 is queued
    ahead of PV of chunk n) so the exp latency is hidden.
  - softmax denominator via a fused ones-column in V (row 64 of the PV
    accumulator).  After the last PV the accumulators are drained to
    SBUF immediately (frees the PSUM banks for the next block); the
    renorm chain (DVE reciprocal_approx_fast + gpsimd
    partition_broadcast + DVE multiply) runs off the critical path at
    raised priority.  reciprocal_approx_fast cannot read PSUM or
    rebase partitions -- plain tensor_copy stages the denominators to
    partition 0 first.
  - q/k projection chains are emitted per i-block just ahead of the
    attention block that consumes them; out-projection blocks trail
    one block behind their renorm.  The final block renormalizes
    t-tile by t-tile with the out-projection interleaved, reading the
    PSUM accumulators directly with a per-head renorm chain, so the
    tail exposure is short.
  - PSUM budget (8 banks): S tiles [128,2,512] x2, PV accumulators
    [65,512] x2, out-proj/warmup [128,512] x2.
"""

import numpy as np
import ml_dtypes

import concourse.bacc as bacc
import concourse.mybir as mybir
import concourse.tile as tile
from concourse.bass_utils import run_bass_kernel_spmd

F32 = mybir.dt.float32
BF16 = mybir.dt.bfloat16

T = 2048            # sequence length
D = 1024            # model dim
DH = 64             # head dim
HPC = 4             # heads per core
NCORES = 8
NTT = T // 128      # 16 t-tiles of 128
NDC = D // 128      # 8 d-chunks of 128
NIB = T // 512      # 4 i-blocks of 512
JPB = 512 // 128    # j-chunks per i-block
BIG = 30000.0


def _build():
    nc = bacc.Bacc("TRN2", target_bir_lowering=False, debug=False)

    XT = nc.dram_tensor("XT", [D, T], BF16, kind="ExternalInput")
    WQKV = nc.dram_tensor("WQKV", [D, 768], BF16, kind="ExternalInput")
    WO = nc.dram_tensor("WO", [256, D], BF16, kind="ExternalInput")
    MASKT = nc.dram_tensor("MASKT", [128, 128], BF16, kind="ExternalInput")
    IDENT = nc.dram_tensor("IDENT", [128, 128], BF16, kind="ExternalInput")
    OUT = nc.dram_tensor("OUT", [T, D], BF16, kind="ExternalOutput")

    EXP = mybir.ActivationFunctionType.Exp

    with tile.TileContext(nc, linearize=True) as tc:
        with tc.tile_pool(name="persist", bufs=1) as pp, \
             tc.tile_pool(name="pt", bufs=4) as ppt, \
             tc.tile_pool(name="prec", bufs=2) as prec, \
             tc.tile_pool(name="pyc", bufs=4) as pyc, \
             tc.tile_pool(name="pbc", bufs=2) as pbc, \
             tc.tile_pool(name="post", bufs=3) as post, \
             tc.tile_pool(name="psS", bufs=2, space="PSUM") as psS, \
             tc.tile_pool(name="psY", bufs=2, space="PSUM") as psY, \
             tc.tile_pool(name="psO", bufs=2, space="PSUM") as psO:

            xt = pp.tile([128, NDC, T], BF16, tag="xt")
            wsb = pp.tile([128, NDC, 768], BF16, tag="wsb")
            wo_sb = pp.tile([128, 2, D], BF16, tag="wo")
            qt = [pp.tile([128, T], BF16, tag=f"qt{p}", name=f"qt{p}")
                  for p in range(2)]
            kt = [pp.tile([128, T], BF16, tag=f"kt{p}", name=f"kt{p}")
                  for p in range(2)]
            vones = pp.tile([128, NTT, HPC, DH + 1], BF16, tag="vones")
            ypair = [pp.tile([128, T], BF16, tag=f"yp{p}", name=f"yp{p}")
                     for p in range(2)]
            maskt = pp.tile([128, 128], BF16, tag="maskt")
            ident = pp.tile([128, 128], BF16, tag="ident")

            # ---- input DMAs ----
            # x is pre-transposed on the host, so every load is a plain
            # contiguous DMA; spread across the two HWDGE queues.
            nc.sync.dma_start(maskt[:], MASKT[:])
            nc.sync.dma_start(ident[:], IDENT[:])
            nc.gpsimd.memset(vones[:, :, :, DH:DH + 1], 1.0)
            xq = [nc.sync, nc.scalar, nc.sync, nc.scalar, nc.gpsimd,
                  nc.sync, nc.scalar, nc.gpsimd]
            for dc in range(NDC):
                nc.gpsimd.dma_start(
                    wsb[:, dc, :],
                    WQKV[dc * 128:(dc + 1) * 128, :])
                xq[dc].dma_start(
                    xt[:, dc, :], XT[dc * 128:(dc + 1) * 128, :])
            for pi in range(2):
                nc.scalar.dma_start(
                    wo_sb[:, pi, :], WO[pi * 128:(pi + 1) * 128, :])

            # PE warm-up: dense dummy matmuls while the input DMAs
            # stream, so HAM un-throttles before the real work arrives.
            warm = psO.tile([128, 512], F32, tag="op", name="warm")
            for _ in range(56):
                nc.tensor.matmul(
                    warm[:, 0:128], maskt[:], ident[:],
                    start=True, stop=True)

            # ---- phase A helpers ----
            def emit_qk(pi, ib):
                for base, dst in ((0, qt[pi]), (256, kt[pi])):
                    qp = psS.tile([128, 512], F32, tag="stab")
                    for dc in range(NDC):
                        nc.tensor.matmul(
                            qp[:],
                            wsb[:, dc, base + pi * 128:base + (pi + 1) * 128],
                            xt[:, dc, ib * 512:(ib + 1) * 512],
                            start=(dc == 0), stop=(dc == NDC - 1))
                    nc.vector.tensor_copy(
                        dst[:, ib * 512:(ib + 1) * 512], qp[:])

            def emit_v(t0, t1):
                for ti in range(t0, t1):
                    vp = psS.tile([128, 256], F32, tag="stab")
                    for dc in range(NDC):
                        nc.tensor.matmul(
                            vp[:], xt[:, dc, ti * 128:(ti + 1) * 128],
                            wsb[:, dc, 512:768],
                            start=(dc == 0), stop=(dc == NDC - 1))
                    nc.vector.tensor_copy(
                        vones[:, ti, :, 0:DH],
                        vp[:].rearrange("p (h d) -> p h d", h=HPC))

            # ---- phase B block: attention for (ib, pi) ----
            def emit_attn(ib, pi, tail=None, pre_renorm=None):
                ya = psY.tile([65, 512], F32, tag="y", name="ya")
                yb = psY.tile([65, 512], F32, tag="y", name="yb")
                njc = JPB * ib + JPB

                def emit_pv(ptab, jc, off):
                    nc.tensor.matmul(
                        ya[0:65, off:512], vones[:, jc, 2 * pi, :],
                        ptab[:, 0, off:512],
                        start=(jc == 0), stop=(jc == njc - 1),
                        skip_group_check=True)
                    nc.tensor.matmul(
                        yb[0:65, off:512], vones[:, jc, 2 * pi + 1, :],
                        ptab[:, 1, off:512],
                        start=(jc == 0), stop=(jc == njc - 1),
                        skip_group_check=True)
                pending = None
                for jc in range(njc):
                    diag = jc >= JPB * ib
                    off = 128 * (jc - JPB * ib) if diag else 0
                    stab = psS.tile([128, 2, 512], F32, tag="stab")
                    ptab = ppt.tile([128, 2, 512], BF16, tag="ptab")
                    js = slice(jc * 128, (jc + 1) * 128)
                    isl = slice(ib * 512 + off, (ib + 1) * 512)
                    nc.tensor.matmul(
                        stab[:, 0, off:512], kt[pi][0:64, js],
                        qt[pi][0:64, isl], start=True, stop=not diag,
                        skip_group_check=True)
                    nc.tensor.matmul(
                        stab[:, 1, off:512], kt[pi][64:128, js],
                        qt[pi][64:128, isl], start=True, stop=not diag,
                        tile_position=(64, 0), skip_group_check=True)
                    if diag:
                        nc.tensor.matmul(
                            stab[:, 0, off:off + 128], maskt[:], ident[:],
                            start=False, stop=True, skip_group_check=True)
                        nc.tensor.matmul(
                            stab[:, 1, off:off + 128], maskt[:], ident[:],
                            start=False, stop=True, skip_group_check=True)
                    nc.scalar.activation(
                        ptab[:, :, off:512], stab[:, :, off:512],
                        EXP, scale=0.125)
                    if pending is not None:
                        emit_pv(*pending)
                    pending = (ptab, jc, off)
                emit_pv(*pending)
                # renorm: y /= denom (row 64), both heads at once
                if pre_renorm is not None:
                    pre_renorm()
                den_sb = prec.tile([1, 2, 512], F32, tag="den")
                rec = prec.tile([1, 2, 512], F32, tag="rec")
                bcs = pbc.tile([64, 2, 512], F32, tag="bcs")
                if tail is None:
                    # drain the PV accumulators to SBUF right away so the
                    # PSUM banks free for the next block; the renorm chain
                    # then runs entirely off the SBUF copy.  High priority
                    # so these DVE ops outrank queued out-proj copies.
                    yca = pyc.tile([65, 512], F32, tag="yc", name="yca")
                    ycb = pyc.tile([65, 512], F32, tag="yc", name="ycb")
                    with tc.high_priority(offset=60):
                        nc.vector.tensor_copy(yca[:], ya[:])
                        nc.vector.tensor_copy(ycb[:], yb[:])
                        nc.vector.tensor_copy(den_sb[:, 0, :], yca[64:65, :])
                        nc.vector.tensor_copy(den_sb[:, 1, :], ycb[64:65, :])
                        nc.vector.reciprocal_approx_fast(rec[:], den_sb[:])
                        nc.gpsimd.partition_broadcast(bcs[:], rec[:])
                else:
                    # final block: latency-critical tail.  Read the PSUM
                    # accumulators directly and run a per-head chain so
                    # head a's reciprocal/broadcast overlaps head b's
                    # last PV matmuls.
                    yca, ycb = ya, yb
                    nc.vector.tensor_copy(den_sb[:, 0, :], yca[64:65, :])
                    nc.vector.reciprocal_approx_fast(
                        rec[:, 0, :], den_sb[:, 0, :])
                    nc.gpsimd.partition_broadcast(
                        bcs[0:64, 0, :], rec[:, 0, :])
                    nc.vector.tensor_copy(den_sb[:, 1, :], ycb[64:65, :])
                    nc.vector.reciprocal_approx_fast(
                        rec[:, 1, :], den_sb[:, 1, :])
                    nc.gpsimd.partition_broadcast(
                        bcs[0:64, 1, :], rec[:, 1, :])
                ibs = slice(ib * 512, (ib + 1) * 512)
                if tail is None:
                    nc.vector.tensor_mul(
                        ypair[pi][0:64, ibs], yca[0:64, :], bcs[0:64, 0, :])
                    nc.vector.tensor_mul(
                        ypair[pi][64:128, ibs], ycb[0:64, :], bcs[0:64, 1, :])
                else:
                    # last block: renormalize t-tile by t-tile and start the
                    # out-projection for each tile as soon as it is ready.
                    for k in range(JPB):
                        ts_ = slice(k * 128, (k + 1) * 128)
                        ti = JPB * ib + k
                        tg = slice(ti * 128, (ti + 1) * 128)
                        nc.vector.tensor_mul(
                            ypair[pi][0:64, tg], yca[0:64, ts_],
                            bcs[0:64, 0, ts_])
                        nc.vector.tensor_mul(
                            ypair[pi][64:128, tg], ycb[0:64, ts_],
                            bcs[0:64, 1, ts_])
                        tail(ti)

            # ---- phase C block: out-projection for i-block ib ----
            def emit_outproj_ti(ti):
                ost = post.tile([128, D], BF16, tag="ost")
                for eh in range(2):
                    op = psO.tile([128, 512], F32, tag="op")
                    nc.tensor.matmul(
                        op[:], ypair[0][:, ti * 128:(ti + 1) * 128],
                        wo_sb[:, 0, eh * 512:(eh + 1) * 512],
                        start=True, stop=False)
                    nc.tensor.matmul(
                        op[:], ypair[1][:, ti * 128:(ti + 1) * 128],
                        wo_sb[:, 1, eh * 512:(eh + 1) * 512],
                        start=False, stop=True)
                    nc.vector.tensor_copy(
                        ost[:, eh * 512:(eh + 1) * 512], op[:])
                nc.sync.dma_start(OUT[ti * 128:(ti + 1) * 128, :], ost[:])

            def emit_outproj(ib):
                for ti in range(JPB * ib, JPB * ib + JPB):
                    ost = post.tile([128, D], BF16, tag="ost")
                    for eh in range(2):
                        op = psO.tile([128, 512], F32, tag="op")
                        nc.tensor.matmul(
                            op[:], ypair[0][:, ti * 128:(ti + 1) * 128],
                            wo_sb[:, 0, eh * 512:(eh + 1) * 512],
                            start=True, stop=False)
                        nc.tensor.matmul(
                            op[:], ypair[1][:, ti * 128:(ti + 1) * 128],
                            wo_sb[:, 1, eh * 512:(eh + 1) * 512],
                            start=False, stop=True)
                        nc.vector.tensor_copy(
                            ost[:, eh * 512:(eh + 1) * 512], op[:])
                    nc.sync.dma_start(
                        OUT[ti * 128:(ti + 1) * 128, :], ost[:])

            # ---- emission schedule ----
            # Fine-grained interleave: q/k projection pairs are emitted
            # per i-block just ahead of the attention block that consumes
            # them, keeping the ACT exp stream fed from ~25us on while the
            # PE works through the projection backlog.
            emit_qk(0, 0)
            emit_v(0, 4)
            emit_attn(0, 0)
            emit_qk(0, 1)
            emit_v(4, 8)
            emit_attn(1, 0)
            emit_qk(1, 0)
            emit_attn(0, 1)
            emit_qk(0, 2)
            emit_v(8, 12)
            emit_attn(2, 0)
            emit_qk(1, 1)
            emit_attn(1, 1)
            with tc.high_priority(offset=-150):
                emit_outproj(0)
            emit_qk(0, 3)
            emit_v(12, 16)
            emit_attn(3, 0)
            emit_qk(1, 2)
            emit_attn(2, 1)
            with tc.high_priority(offset=-150):
                emit_outproj(1)
            emit_qk(1, 3)
            with tc.high_priority(offset=-150):
                for _ti in range(8, 10):
                    emit_outproj_ti(_ti)
            emit_attn(3, 1, tail=emit_outproj_ti,
                      pre_renorm=lambda: [emit_outproj_ti(10),
                                          emit_outproj_ti(11)])

    nc.compile()
    return nc


_NC = None


def build_in_maps(x, w_qkv, w_out):
    x = np.asarray(x, np.float32)
    w_qkv = np.asarray(w_qkv, np.float32)
    w_out = np.asarray(w_out, np.float32)

    idx = np.arange(128)
    maskt = np.where(idx[None, :] > idx[:, None], -BIG, 0.0).astype(
        ml_dtypes.bfloat16)                       # maskt[i,j] = -BIG iff j>i
    identm = np.eye(128, dtype=ml_dtypes.bfloat16)

    in_maps = []
    for c in range(NCORES):
        b, g = divmod(c, 4)
        cs = slice(g * 256, (g + 1) * 256)
        wq = w_qkv[:, 0 * 1024:1 * 1024][:, cs]
        wk = w_qkv[:, 1 * 1024:2 * 1024][:, cs]
        wv = w_qkv[:, 2 * 1024:3 * 1024][:, cs]
        wqkv = np.ascontiguousarray(
            np.concatenate([wq, wk, wv], axis=1)).astype(ml_dtypes.bfloat16)
        wo = np.ascontiguousarray(
            w_out[g * 256:(g + 1) * 256, :]).astype(ml_dtypes.bfloat16)
        in_maps.append({
            "XT": np.ascontiguousarray(x[b].T).astype(ml_dtypes.bfloat16),
            "WQKV": wqkv,
            "WO": wo,
            "MASKT": maskt,
            "IDENT": identm,
        })
    return in_maps


def kernel(x, w_qkv, w_out):
    global _NC
    if _NC is None:
        _NC = _build()

    in_maps = build_in_maps(x, w_qkv, w_out)
    res = run_bass_kernel_spmd(_NC, in_maps, core_ids=list(range(NCORES)))
    outs = [res.results[c]["OUT"].astype(np.float32) for c in range(NCORES)]
    y = np.stack([outs[0] + outs[1] + outs[2] + outs[3],
                  outs[4] + outs[5] + outs[6] + outs[7]], axis=0)
    return y.astype(np.float32)


# revision 38
# speedup vs baseline: 2.8296x; 2.8296x over previous
"""Causal self-attention (b=2, t=2048, d=1024, h=16) on 8 trn2 NeuronCores.

Sharding: core c handles batch c//4 and the 4 heads 4*(c%4)..4*(c%4)+3
(data parallel over batch x tensor parallel over heads). Each core
computes x @ w_qkv for its head-slice, causal attention for its heads,
and a partial out-projection y_heads @ w_out[head_rows]; the host sums
the 4 partial bf16 outputs per batch (the tensor-parallel all-reduce).

Design (HW ~184-187us vs ~395us f32r baseline; rel err 5.6e-3, gate 2e-2):
  - bf16 end-to-end; x is pre-transposed on the host so every input
    load is a plain contiguous DMA, spread over the sync/scalar HWDGE
    queues plus the gpsimd SWDGE queue (transpose-mode DMAs on two
    queues raced and corrupted SBUF; all-normal-mode is safe).
  - dummy maskt@I matmuls bridge the initial DMA wait so the PE HAM
    clock-gate is warm (2.4 GHz) when real work arrives.
  - causal mask folded into the S PSUM accumulation as a matmul with a
    constant -BIG upper-triangle (maskT @ I), so exp(scale*(S+mask))=0
    above the diagonal -- no vector-engine masking pass.
  - one fused exp per j-chunk over both heads ([128, 2, 512] PSUM tile);
    the PE emission is software-pipelined (S of chunk n+1 is queued
    ahead of PV of chunk n) so the exp latency is hidden.
  - softmax denominator via a fused ones-column in V (row 64 of the PV
    accumulator).  After the last PV the accumulators are drained to
    SBUF immediately (frees the PSUM banks for the next block); the
    renorm chain (DVE reciprocal_approx_fast + gpsimd
    partition_broadcast + DVE multiply) runs off the critical path at
    raised priority.  reciprocal_approx_fast cannot read PSUM or
    rebase partitions -- plain tensor_copy stages the denominators to
    partition 0 first.
  - q/k projection chains are emitted per i-block just ahead of the
    attention block that consumes them; out-projection blocks trail
    one block behind their renorm.  The final block renormalizes
    t-tile by t-tile with the out-projection interleaved, reading the
    PSUM accumulators directly with a per-head renorm chain, so the
    tail exposure is short.
  - PSUM budget (8 banks): S tiles [128,2,512] x2, PV accumulators
    [65,512] x2, out-proj/warmup [128,512] x2.
"""

import numpy as np
import ml_dtypes

import concourse.bacc as bacc
import concourse.mybir as mybir
import concourse.tile as tile
from concourse.bass_utils import run_bass_kernel_spmd

F32 = mybir.dt.float32
BF16 = mybir.dt.bfloat16

T = 2048            # sequence length
D = 1024            # model dim
DH = 64             # head dim
HPC = 4             # heads per core
NCORES = 8
NTT = T // 128      # 16 t-tiles of 128
NDC = D // 128      # 8 d-chunks of 128
NIB = T // 512      # 4 i-blocks of 512
JPB = 512 // 128    # j-chunks per i-block
BIG = 30000.0


def _build():
    nc = bacc.Bacc("TRN2", target_bir_lowering=False, debug=False)

    XT = nc.dram_tensor("XT", [D, T], BF16, kind="ExternalInput")
    WQKV = nc.dram_tensor("WQKV", [D, 768], BF16, kind="ExternalInput")
    WO = nc.dram_tensor("WO", [256, D], BF16, kind="ExternalInput")
    MASKT = nc.dram_tensor("MASKT", [128, 128], BF16, kind="ExternalInput")
    IDENT = nc.dram_tensor("IDENT", [128, 128], BF16, kind="ExternalInput")
    OUT = nc.dram_tensor("OUT", [T, D], BF16, kind="ExternalOutput")

    EXP = mybir.ActivationFunctionType.Exp

    with tile.TileContext(nc) as tc:
        with tc.tile_pool(name="persist", bufs=1) as pp, \
             tc.tile_pool(name="pt", bufs=4) as ppt, \
             tc.tile_pool(name="prec", bufs=2) as prec, \
             tc.tile_pool(name="pyc", bufs=4) as pyc, \
             tc.tile_pool(name="pbc", bufs=2) as pbc, \
             tc.tile_pool(name="post", bufs=3) as post, \
             tc.tile_pool(name="psS", bufs=2, space="PSUM") as psS, \
             tc.tile_pool(name="psY", bufs=2, space="PSUM") as psY, \
             tc.tile_pool(name="psO", bufs=2, space="PSUM") as psO:

            xt = pp.tile([128, NDC, T], BF16, tag="xt")
            wsb = pp.tile([128, NDC, 768], BF16, tag="wsb")
            wo_sb = pp.tile([128, 2, D], BF16, tag="wo")
            qt = [pp.tile([128, T], BF16, tag=f"qt{p}", name=f"qt{p}")
                  for p in range(2)]
            kt = [pp.tile([128, T], BF16, tag=f"kt{p}", name=f"kt{p}")
                  for p in range(2)]
            vones = pp.tile([128, NTT, HPC, DH + 1], BF16, tag="vones")
            ypair = [pp.tile([128, T], BF16, tag=f"yp{p}", name=f"yp{p}")
                     for p in range(2)]
            maskt = pp.tile([128, 128], BF16, tag="maskt")
            ident = pp.tile([128, 128], BF16, tag="ident")

            # ---- input DMAs ----
            # x is pre-transposed on the host, so every load is a plain
            # contiguous DMA; spread across the two HWDGE queues.
            nc.sync.dma_start(maskt[:], MASKT[:])
            nc.sync.dma_start(ident[:], IDENT[:])
            nc.gpsimd.memset(vones[:, :, :, DH:DH + 1], 1.0)
            xq = [nc.sync, nc.scalar, nc.sync, nc.scalar, nc.gpsimd,
                  nc.sync, nc.scalar, nc.gpsimd]
            for dc in range(NDC):
                nc.gpsimd.dma_start(
                    wsb[:, dc, :],
                    WQKV[dc * 128:(dc + 1) * 128, :])
                xq[dc].dma_start(
                    xt[:, dc, :], XT[dc * 128:(dc + 1) * 128, :])
            for pi in range(2):
                nc.scalar.dma_start(
                    wo_sb[:, pi, :], WO[pi * 128:(pi + 1) * 128, :])

            # PE warm-up: dense dummy matmuls while the input DMAs
            # stream, so HAM un-throttles before the real work arrives.
            warm = psO.tile([128, 512], F32, tag="op", name="warm")
            for _ in range(56):
                nc.tensor.matmul(
                    warm[:, 0:128], maskt[:], ident[:],
                    start=True, stop=True)

            # ---- phase A helpers ----
            def emit_qk(pi, ib):
                for base, dst in ((0, qt[pi]), (256, kt[pi])):
                    qp = psS.tile([128, 512], F32, tag="stab")
                    for dc in range(NDC):
                        nc.tensor.matmul(
                            qp[:],
                            wsb[:, dc, base + pi * 128:base + (pi + 1) * 128],
                            xt[:, dc, ib * 512:(ib + 1) * 512],
                            start=(dc == 0), stop=(dc == NDC - 1))
                    nc.vector.tensor_copy(
                        dst[:, ib * 512:(ib + 1) * 512], qp[:])

            def emit_v(t0, t1):
                for ti in range(t0, t1):
                    vp = psS.tile([128, 256], F32, tag="stab")
                    for dc in range(NDC):
                        nc.tensor.matmul(
                            vp[:], xt[:, dc, ti * 128:(ti + 1) * 128],
                            wsb[:, dc, 512:768],
                            start=(dc == 0), stop=(dc == NDC - 1))
                    nc.vector.tensor_copy(
                        vones[:, ti, :, 0:DH],
                        vp[:].rearrange("p (h d) -> p h d", h=HPC))

            # ---- phase B block: attention for (ib, pi) ----
            def emit_attn(ib, pi, tail=None, pre_renorm=None):
                ya = psY.tile([65, 512], F32, tag="y", name="ya")
                yb = psY.tile([65, 512], F32, tag="y", name="yb")
                njc = JPB * ib + JPB

                def emit_pv(ptab, jc, off):
                    nc.tensor.matmul(
                        ya[0:65, off:512], vones[:, jc, 2 * pi, :],
                        ptab[:, 0, off:512],
                        start=(jc == 0), stop=(jc == njc - 1),
                        skip_group_check=True)
                    nc.tensor.matmul(
                        yb[0:65, off:512], vones[:, jc, 2 * pi + 1, :],
                        ptab[:, 1, off:512],
                        start=(jc == 0), stop=(jc == njc - 1),
                        skip_group_check=True)
                pending = None
                for jc in range(njc):
                    diag = jc >= JPB * ib
                    off = 128 * (jc - JPB * ib) if diag else 0
                    stab = psS.tile([128, 2, 512], F32, tag="stab")
                    ptab = ppt.tile([128, 2, 512], BF16, tag="ptab")
                    js = slice(jc * 128, (jc + 1) * 128)
                    isl = slice(ib * 512 + off, (ib + 1) * 512)
                    nc.tensor.matmul(
                        stab[:, 0, off:512], kt[pi][0:64, js],
                        qt[pi][0:64, isl], start=True, stop=not diag,
                        skip_group_check=True)
                    nc.tensor.matmul(
                        stab[:, 1, off:512], kt[pi][64:128, js],
                        qt[pi][64:128, isl], start=True, stop=not diag,
                        tile_position=(64, 0), skip_group_check=True)
                    if diag:
                        nc.tensor.matmul(
                            stab[:, 0, off:off + 128], maskt[:], ident[:],
                            start=False, stop=True, skip_group_check=True)
                        nc.tensor.matmul(
                            stab[:, 1, off:off + 128], maskt[:], ident[:],
                            start=False, stop=True, skip_group_check=True)
                    nc.scalar.activation(
                        ptab[:, :, off:512], stab[:, :, off:512],
                        EXP, scale=0.125)
                    if pending is not None:
                        emit_pv(*pending)
                    pending = (ptab, jc, off)
                emit_pv(*pending)
                # renorm: y /= denom (row 64), both heads at once
                if pre_renorm is not None:
                    pre_renorm()
                den_sb = prec.tile([1, 2, 512], F32, tag="den")
                rec = prec.tile([1, 2, 512], F32, tag="rec")
                bcs = pbc.tile([64, 2, 512], F32, tag="bcs")
                if tail is None:
                    # drain the PV accumulators to SBUF right away so the
                    # PSUM banks free for the next block; the renorm chain
                    # then runs entirely off the SBUF copy.  High priority
                    # so these DVE ops outrank queued out-proj copies.
                    yca = pyc.tile([65, 512], F32, tag="yc", name="yca")
                    ycb = pyc.tile([65, 512], F32, tag="yc", name="ycb")
                    with tc.high_priority(offset=60):
                        nc.vector.tensor_copy(yca[:], ya[:])
                        nc.vector.tensor_copy(ycb[:], yb[:])
                        nc.vector.tensor_copy(den_sb[:, 0, :], yca[64:65, :])
                        nc.vector.tensor_copy(den_sb[:, 1, :], ycb[64:65, :])
                        nc.vector.reciprocal_approx_fast(rec[:], den_sb[:])
                        nc.gpsimd.partition_broadcast(bcs[:], rec[:])
                else:
                    # final block: latency-critical tail.  Read the PSUM
                    # accumulators directly and run a per-head chain so
                    # head a's reciprocal/broadcast overlaps head b's
                    # last PV matmuls.
                    yca, ycb = ya, yb
                    nc.vector.tensor_copy(den_sb[:, 0, :], yca[64:65, :])
                    nc.vector.reciprocal_approx_fast(
                        rec[:, 0, :], den_sb[:, 0, :])
                    nc.gpsimd.partition_broadcast(
                        bcs[0:64, 0, :], rec[:, 0, :])
                    nc.vector.tensor_copy(den_sb[:, 1, :], ycb[64:65, :])
                    nc.vector.reciprocal_approx_fast(
                        rec[:, 1, :], den_sb[:, 1, :])
                    nc.gpsimd.partition_broadcast(
                        bcs[0:64, 1, :], rec[:, 1, :])
                ibs = slice(ib * 512, (ib + 1) * 512)
                if tail is None:
                    nc.vector.tensor_mul(
                        ypair[pi][0:64, ibs], yca[0:64, :], bcs[0:64, 0, :])
                    nc.vector.tensor_mul(
                        ypair[pi][64:128, ibs], ycb[0:64, :], bcs[0:64, 1, :])
                else:
                    # last block: renormalize t-tile by t-tile and start the
                    # out-projection for each tile as soon as it is ready.
                    for k in range(JPB):
                        ts_ = slice(k * 128, (k + 1) * 128)
                        ti = JPB * ib + k
                        tg = slice(ti * 128, (ti + 1) * 128)
                        nc.vector.tensor_mul(
                            ypair[pi][0:64, tg], yca[0:64, ts_],
                            bcs[0:64, 0, ts_])
                        nc.vector.tensor_mul(
                            ypair[pi][64:128, tg], ycb[0:64, ts_],
                            bcs[0:64, 1, ts_])
                        tail(ti)

            # ---- phase C block: out-projection for i-block ib ----
            def emit_outproj_ti(ti):
                ost = post.tile([128, D], BF16, tag="ost")
                for eh in range(2):
                    op = psO.tile([128, 512], F32, tag="op")
                    nc.tensor.matmul(
                        op[:], ypair[0][:, ti * 128:(ti + 1) * 128],
                        wo_sb[:, 0, eh * 512:(eh + 1) * 512],
                        start=True, stop=False)
                    nc.tensor.matmul(
                        op[:], ypair[1][:, ti * 128:(ti + 1) * 128],
                        wo_sb[:, 1, eh * 512:(eh + 1) * 512],
                        start=False, stop=True)
                    nc.vector.tensor_copy(
                        ost[:, eh * 512:(eh + 1) * 512], op[:])
                nc.sync.dma_start(OUT[ti * 128:(ti + 1) * 128, :], ost[:])

            def emit_outproj(ib):
                for ti in range(JPB * ib, JPB * ib + JPB):
                    ost = post.tile([128, D], BF16, tag="ost")
                    for eh in range(2):
                        op = psO.tile([128, 512], F32, tag="op")
                        nc.tensor.matmul(
                            op[:], ypair[0][:, ti * 128:(ti + 1) * 128],
                            wo_sb[:, 0, eh * 512:(eh + 1) * 512],
                            start=True, stop=False)
                        nc.tensor.matmul(
                            op[:], ypair[1][:, ti * 128:(ti + 1) * 128],
                            wo_sb[:, 1, eh * 512:(eh + 1) * 512],
                            start=False, stop=True)
                        nc.vector.tensor_copy(
                            ost[:, eh * 512:(eh + 1) * 512], op[:])
                    nc.sync.dma_start(
                        OUT[ti * 128:(ti + 1) * 128, :], ost[:])

            # ---- emission schedule ----
            # Fine-grained interleave: q/k projection pairs are emitted
            # per i-block just ahead of the attention block that consumes
            # them, keeping the ACT exp stream fed from ~25us on while the
            # PE works through the projection backlog.
            emit_qk(0, 0)
            emit_v(0, 4)
            emit_attn(0, 0)
            emit_qk(0, 1)
            emit_v(4, 8)
            emit_attn(1, 0)
            emit_qk(1, 0)
            emit_attn(0, 1)
            emit_qk(0, 2)
            emit_v(8, 12)
            emit_attn(2, 0)
            emit_qk(1, 1)
            emit_attn(1, 1)
            with tc.high_priority(offset=-150):
                emit_outproj(0)
            emit_qk(0, 3)
            emit_v(12, 16)
            emit_attn(3, 0)
            emit_qk(1, 2)
            emit_attn(2, 1)
            with tc.high_priority(offset=-150):
                emit_outproj(1)
            emit_qk(1, 3)
            with tc.high_priority(offset=-150):
                for _ti in range(8, 10):
                    emit_outproj_ti(_ti)
            emit_attn(3, 1, tail=emit_outproj_ti,
                      pre_renorm=lambda: [emit_outproj_ti(10),
                                          emit_outproj_ti(11)])

    nc.compile()
    return nc


_NC = None


def build_in_maps(x, w_qkv, w_out):
    x = np.asarray(x, np.float32)
    w_qkv = np.asarray(w_qkv, np.float32)
    w_out = np.asarray(w_out, np.float32)

    idx = np.arange(128)
    maskt = np.where(idx[None, :] > idx[:, None], -BIG, 0.0).astype(
        ml_dtypes.bfloat16)                       # maskt[i,j] = -BIG iff j>i
    identm = np.eye(128, dtype=ml_dtypes.bfloat16)

    in_maps = []
    for c in range(NCORES):
        b, g = divmod(c, 4)
        cs = slice(g * 256, (g + 1) * 256)
        wq = w_qkv[:, 0 * 1024:1 * 1024][:, cs]
        wk = w_qkv[:, 1 * 1024:2 * 1024][:, cs]
        wv = w_qkv[:, 2 * 1024:3 * 1024][:, cs]
        wqkv = np.ascontiguousarray(
            np.concatenate([wq, wk, wv], axis=1)).astype(ml_dtypes.bfloat16)
        wo = np.ascontiguousarray(
            w_out[g * 256:(g + 1) * 256, :]).astype(ml_dtypes.bfloat16)
        in_maps.append({
            "XT": np.ascontiguousarray(x[b].T).astype(ml_dtypes.bfloat16),
            "WQKV": wqkv,
            "WO": wo,
            "MASKT": maskt,
            "IDENT": identm,
        })
    return in_maps


def kernel(x, w_qkv, w_out):
    global _NC
    if _NC is None:
        _NC = _build()

    in_maps = build_in_maps(x, w_qkv, w_out)
    res = run_bass_kernel_spmd(_NC, in_maps, core_ids=list(range(NCORES)))
    outs = [res.results[c]["OUT"].astype(np.float32) for c in range(NCORES)]
    y = np.stack([outs[0] + outs[1] + outs[2] + outs[3],
                  outs[4] + outs[5] + outs[6] + outs[7]], axis=0)
    return y.astype(np.float32)
